# revision 5
# baseline (speedup 1.0000x reference)
"""CenterLoss Trainium2 kernel (fp8 DoubleRow, on-device squaring).

Full inputs:
  ep_mask_embed    (8, 4096, 256) f32
  ep_mask          (8, 1, 1024, 1024) f32
  query_mask_embed (8, 4096, 256) f32
  query_mask       (8, 1, 1024, 1024) f32
Output: (3,) f32 = [mean(center_loss), mean(pos_loss), mean(neg_loss)]

Sharding: data-parallel, one batch sample per NeuronCore (8 cores).

Per sample the loss reduces to epw = [m;1-m]^T ep, qw = [m;1-m]^T q,
qsqw = [m;1-m]^T q^2 plus mask counts; everything downstream is ~50
scalar flops done on host from those statistics (where the batch mean
already happens).

Trace-driven structure (v3):
  - HBM traffic 2MB/core (not 3MB): q^2 is squared on-device instead
    of host-precomputed+streamed.  DVE and ACT (both idle otherwise)
    split each quarter-chunk of q by columns, balanced by their
    errata-adjusted rates ((151+x)/0.96 vs (224+2048-x)/1.2).
  - q streams first as 2x256KB then 1x512KB so squaring starts ~1.4us
    earlier than a single 512KB q0 (completion sems fire per-DMA);
    ep follows as 2x512KB (4KB/partition descriptors = full ~340GB/s).
  - All stream DMAs ride the SYNC HWDGE ring in consumption order;
    the ACT ring stays free so the ~2.7us Square ACT_TABLE_LOAD hides
    under the DMA ramp.
  - PE warm-up: zero matmuls during the DMA ramp flip the HAM clock
    gate (4/8 -> 8/8) so real matmuls run ~109ns instead of 213ns.
  - fp8 squares are scaled by sqrt(2) on device (divided back out on
    host): squares of fp8 grid points land just above RNE midpoints,
    giving a systematic ~-0.5% bias when re-rounded to fp8; the
    irrational scale decorrelates the rounding (rel err 7.7e-3 -> ~1e-3).
  - Single out-DMA for all three stat sections.
"""

import numpy as np
import ml_dtypes
from contextlib import ExitStack

import concourse.bass as bass
import concourse.bacc as bacc
import concourse.tile as tile
from concourse import mybir
from concourse.bass_utils import run_bass_kernel_spmd

F32 = mybir.dt.float32
F8 = mybir.dt.float8e4
NP_F8 = ml_dtypes.float8_e4m3fn

P = 128          # partitions
N_TOK = 4096     # tokens per sample (64*64 patches)
C = 256          # channels
T = 16           # tokens per partition per chunk (4KB fp8 descriptor)
DC = P * T       # tokens per chunk (2048)
N_DC = N_TOK // DC   # 2 chunks
NPC = T // 2     # parity-pairs (pieces) per chunk: 8
B = 8            # batch == n cores
PATCH = 16
QTR = 2048       # square-granularity: quarter of the q stream (cols)
SQ_SPLIT = 926   # within each quarter: DVE squares [0:926), ACT rest
SQ_SCALE = float(np.sqrt(2.0))
SQ_SCALE_ACT = float(2.0 ** 0.25)   # applied inside Square => sqrt(2)
N_WARM = 13      # PE warm-up matmuls (~2.8us of the DMA ramp)

_CACHE = {}


def _build():
    """Build the per-core Bass program (identical on all cores)."""
    nc = bacc.Bacc("TRN2", target_bir_lowering=False, debug=False)

    ep8 = nc.dram_tensor("ep8", [N_TOK, C], F8, kind="ExternalInput").ap()
    q8 = nc.dram_tensor("q8", [N_TOK, C], F8, kind="ExternalInput").ap()
    # host-packed DoubleRow mask weights.  The dual-fp8 ldweights ISA
    # check needs the dual-row AP dim to have num_elem==2 and a step
    # that is a multiple of 16 elements, so the two ks sub-rows live in
    # separate 64-col planes: col = 64*ks + 4*jj + m,
    # m in (q_pos, q_neg, ep_pos, ep_neg),
    # token = 2048*(jj//8) + 16*p + 2*(jj%8) + ks
    lw = nc.dram_tensor("lw", [P, 8 * N_DC * NPC], F8, kind="ExternalInput").ap()
    # [epw | qw | qsqw], rows = (pos, neg)
    out = nc.dram_tensor("out", [2, 3 * C], F32, kind="ExternalOutput").ap()

    DR = mybir.MatmulPerfMode.DoubleRow

    with tile.TileContext(nc) as tc, ExitStack() as ctx:
        const_pool = ctx.enter_context(tc.tile_pool(name="const", bufs=1))
        x_pool = ctx.enter_context(tc.tile_pool(name="x_pool", bufs=1))
        sq_pool = ctx.enter_context(tc.tile_pool(name="sq_pool", bufs=1))
        psum_pool = ctx.enter_context(
            tc.tile_pool(name="psum", bufs=1, space=bass.MemorySpace.PSUM)
        )
        fin_pool = ctx.enter_context(tc.tile_pool(name="fin", bufs=1))

        lw_t = const_pool.tile([P, 8 * N_DC * NPC], F8, name="lw_t", tag="lw_t")
        nc.sync.dma_start(out=lw_t[:], in_=lw[:])

        # PE warm-up: zeros tile -> N=256 normal-mode matmuls into a
        # scratch PSUM bank.  No stream dependency, so they run during
        # the DMA ramp and flip HAM to 8/8 before real work arrives.
        warm = const_pool.tile([P, C], F8, name="warm", tag="warm")
        nc.gpsimd.memset(warm[:], 0)
        warm_ps = psum_pool.tile([P, C], F32, name="warm_ps", tag="warm_ps")
        for _ in range(N_WARM):
            nc.tensor.matmul(
                warm_ps[:], warm[:, 0:P], warm[:], start=True, stop=True
            )

        # q/ep SBUF tiles (one per 512KB chunk; q chunk 0 is filled by
        # two 256KB DMAs so its completion sems fire earlier).
        X = {}
        for nm, src in (("q", q8), ("ep", ep8)):
            for i in range(N_DC):
                t_ = x_pool.tile([P, T * C], F8, name=f"x{nm}{i}", tag=f"x{nm}{i}")
                X[(nm, i)] = t_

        def stream(nm, i, h0, h1):
            """DMA tokens [h0:h1) of each partition's chunk-i row."""
            src = q8 if nm == "q" else ep8
            nc.sync.dma_start(
                out=X[(nm, i)][:, h0 * C:h1 * C],
                in_=src[i * DC:(i + 1) * DC, :].rearrange(
                    "(p t) c -> p t c", t=T)[:, h0:h1, :],
            )

        # consumption order: qA(256K), qB(256K), q1(512K), ep0, ep1
        stream("q", 0, 0, T // 2)
        stream("q", 0, T // 2, T)
        stream("q", 1, 0, T)
        stream("ep", 0, 0, T)
        stream("ep", 1, 0, T)

        # On-device sqrt(2)*q^2 (fp8 in/out, fp32 internal), one
        # DVE+ACT op pair per quarter as its data lands.
        SQ = {}
        for i in range(N_DC):
            sq = sq_pool.tile([P, T * C], F8, name=f"sq{i}", tag=f"sq{i}")
            qt = X[("q", i)]
            for h in range(2):
                base = h * QTR
                nc.vector.scalar_tensor_tensor(
                    sq[:, base:base + SQ_SPLIT],
                    qt[:, base:base + SQ_SPLIT],
                    SQ_SCALE,
                    qt[:, base:base + SQ_SPLIT],
                    mybir.AluOpType.mult,
                    mybir.AluOpType.mult,
                )
                nc.scalar.activation(
                    sq[:, base + SQ_SPLIT:base + QTR],
                    qt[:, base + SQ_SPLIT:base + QTR],
                    mybir.ActivationFunctionType.Square,
                    scale=SQ_SCALE_ACT,
                )
            SQ[("qsq", i)] = sq

        psum = {
            nm: psum_pool.tile([2, C], F32, name=f"ps_{nm}", tag=f"ps_{nm}")
            for nm in ("ep", "q", "qsq")
        }

        fin = fin_pool.tile([2, 3 * C], F32, name="fin", tag="fin")
        SEC = {"ep": 0, "q": 1, "qsq": 2}
        WOFF = {"ep": 2, "q": 0, "qsq": 0}

        # PE bursts in expected data-availability order.  Each entry:
        # (chain, chunk, first piece, n pieces).
        bursts = [
            ("q", 0, 0, 4), ("q", 0, 4, 4),      # qA, qB
            ("qsq", 0, 0, 4),                     # sqA
            ("qsq", 0, 4, 4),                     # sqB
            ("q", 1, 0, 8),                       # q1
            ("qsq", 1, 0, 4),                     # sqC
            ("ep", 0, 0, 8),
            ("qsq", 1, 4, 4),                     # sqD
            ("ep", 1, 0, 8),
        ]
        for nm, i, j0, nj in bursts:
            tiles = SQ if nm == "qsq" else X
            src_t = tiles[(nm, i)]
            for j in range(j0, j0 + nj):
                jj = NPC * i + j
                off = 4 * jj + WOFF[nm]
                w = lw_t[:].rearrange(
                    "p (k c) -> p k c", k=2)[:, :, off:off + 2]
                rhs = src_t[:, 512 * j:512 * (j + 1)].rearrange(
                    "p (k c) -> p k c", k=2)
                nc.tensor.matmul(
                    psum[nm][:], w, rhs,
                    start=(i == 0 and j == 0),
                    stop=(i == N_DC - 1 and j == NPC - 1),
                    perf_mode=DR,
                )
            if i == N_DC - 1 and j0 + nj == NPC:
                s = SEC[nm]
                # last chain (ep) ships via the by-then-idle DVE; the
                # earlier two via ACT (its squares are done by then).
                fsec = fin[:, s * C:(s + 1) * C]
                if nm == "ep":
                    nc.vector.tensor_copy(fsec, psum[nm][:])
                else:
                    nc.scalar.copy(fsec, psum[nm][:])

        # single out-DMA for all three sections
        nc.sync.dma_start(out=out[:], in_=fin[:])

    nc.compile()
    return nc


def get_nc():
    if "nc" not in _CACHE:
        _CACHE["nc"] = _build()
    return _CACHE["nc"]


# token index per (partition, piece jj, ks): DoubleRow weight layout
_PG = np.arange(P)[:, None, None]
_JJ = np.arange(N_DC * NPC)[None, :, None]
_KS = np.arange(2)[None, None, :]
_TOK = (DC * (_JJ // NPC) + T * _PG + 2 * (_JJ % NPC) + _KS)  # [128, 16, 2]


def _mask_ds(mask_b):
    """Downsample one sample's mask (nearest, stride 16) -> (4096,) f64."""
    return mask_b[0, ::PATCH, ::PATCH].reshape(-1).astype(np.float64)


def make_in_maps(ep_mask_embed, ep_mask, query_mask_embed, query_mask):
    in_maps, counts = [], []
    for b in range(B):
        em = _mask_ds(ep_mask[b])
        qm = _mask_ds(query_mask[b])
        et = em[_TOK]  # [128, 16, 2] = (p, jj, ks)
        qt = qm[_TOK]
        L = np.stack([qt, 1.0 - qt, et, 1.0 - et], axis=-1)  # [p,jj,ks,m]
        lw_b = L.transpose(0, 2, 1, 3)  # [p, ks, jj, m] -> col 64ks+4jj+m
        in_maps.append({
            "ep8": np.ascontiguousarray(ep_mask_embed[b]).astype(NP_F8),
            "q8": np.ascontiguousarray(query_mask_embed[b]).astype(NP_F8),
            "lw": lw_b.reshape(P, 8 * N_DC * NPC).astype(NP_F8),
        })
        counts.append((em.sum(), (1.0 - em).sum(), qm.sum(), (1.0 - qm).sum()))
    return in_maps, counts


def finalize(per_core, counts):
    """per_core: list of 8 arrays [2, 768] (epw|qw|qsqw) -> full (3,)."""
    pos = np.zeros(B)
    neg = np.zeros(B)
    for b in range(B):
        st = np.asarray(per_core[b]).astype(np.float64)
        n_pe, n_ne, n_pq, n_nq = counts[b]
        epw, qw = st[:, 0:C], st[:, C:2 * C]
        qsq = st[:, 2 * C:3 * C] / SQ_SCALE
        pc = epw[0] / (n_pe + 0.1)
        ncen = epw[1] / (n_ne + 0.1)
        pn = qsq[0].sum() - 2.0 * (pc @ qw[0]) + n_pq * (pc @ pc)
        nn = qsq[1].sum() - 2.0 * (ncen @ qw[1]) + n_nq * (ncen @ ncen)
        pos[b] = pn / (max(n_pq, 1.0) * C) if n_pq > 0 else 0.0
        neg[b] = nn / (max(n_nq, 1.0) * C) if n_nq > 0 else 0.0
    return np.array(
        [(pos + neg).mean(), pos.mean(), neg.mean()], dtype=np.float32
    )


def kernel(ep_mask_embed, ep_mask, query_mask_embed, query_mask):
    ep_mask_embed = np.asarray(ep_mask_embed, dtype=np.float32)
    ep_mask = np.asarray(ep_mask, dtype=np.float32)
    query_mask_embed = np.asarray(query_mask_embed, dtype=np.float32)
    query_mask = np.asarray(query_mask, dtype=np.float32)

    nc = get_nc()
    in_maps, counts = make_in_maps(
        ep_mask_embed, ep_mask, query_mask_embed, query_mask)
    # First execution after device bring-up has been observed to return
    # garbage once; retry on non-finite results.
    for _ in range(3):
        res = run_bass_kernel_spmd(nc, in_maps, list(range(B)))
        result = finalize([r["out"] for r in res.results], counts)
        if np.all(np.isfinite(result)):
            return result
    return result
